# revision 15
# baseline (speedup 1.0000x reference)
"""AGCN Trainium2 kernel — 8-core data-parallel over batch.

Math (per batch b, N=1000 nodes, Din=32, Dout=64, D=16, K=2):
  AS  = relu(tanh(3 E E^T))                       [N,N] static, symmetric
  d   = rowsum(AS)^-1/2 ; AG = d AS d
  ho  = AS @ x[b]; DF = MLP(ho); Et = E*TD*TW; DE = tanh(3 Et DF)
  DA  = relu(tanh(3 DE DE^T))                     [N,N] per batch
  out = a*(einsum over per-node weights E@Wpool applied to [x, AG@x]) + a*E@bias_pool
      + b*(DA @ x) @ Wdg

Each core handles 4 batches; AS/AG/weights replicated per core.
Key layouts (per core, nodes padded 1000->1024):
  feature-major "packed" tensors put batch j's features at partitions 32j.
  agconv: y^T[(d,ki),n] = E_or_Ed[d,n] * [x^T;AG@x^T][ki,n] built by one
  tensor_tensor per (batch, d-pair chunk); contraction (d,ki)=1024 on PE.
"""

import os
import sys

for _p in ("/opt/trn_rl_repo", "/root/.axon_site/_ro/trn_rl_repo"):
    if os.path.isdir(_p) and _p not in sys.path:
        sys.path.append(_p)

from contextlib import ExitStack

import ml_dtypes
import numpy as np

import concourse.bass as bass
import concourse.tile as tile
from concourse import bacc, mybir
from concourse.masks import make_identity

BF16 = mybir.dt.bfloat16
F32 = mybir.dt.float32
I32 = mybir.dt.int32
AF = mybir.ActivationFunctionType
OP = mybir.AluOpType
bfloat16 = ml_dtypes.bfloat16

NCORES = 8
NB = 4  # batches per core
N = 1000
NP = 1024
NT = 8  # node tiles of 128
ALPHA = 3.0
RSQRT_MAGIC = 0x5F3759DF

LAST_EXEC_NS = None
_NC_CACHE = {}


def _build_body(nc, tc, ctx, t):
    """t: dict of dram tensor handles."""
    P = 128
    PHASES = int(os.environ.get("KERNEL_PHASES", "6"))

    pers = ctx.enter_context(tc.tile_pool(name="pers", bufs=1))
    work = ctx.enter_context(tc.tile_pool(name="work", bufs=3))
    da_p = ctx.enter_context(tc.tile_pool(name="da", bufs=4))
    yt_p = ctx.enter_context(tc.tile_pool(name="yt", bufs=3))
    # PSUM static budget: 8 banks = 16KB/partition.
    # ps_b tag "big" bufs=2 -> 4 banks: AS, d-chain, MLP, S tiles (sequential uses)
    # ps_a bufs=1, tag acc1 (ho -> dg -> tr) 2 banks, tag acc2 (xg -> ag) 2 banks
    ps_b = ctx.enter_context(tc.tile_pool(name="ps_b", bufs=2, space="PSUM"))
    ps_a = ctx.enter_context(tc.tile_pool(name="ps_a", bufs=1, space="PSUM"))
    dram = ctx.enter_context(tc.tile_pool(name="dram", bufs=1, space="DRAM"))

    # ---- persistent SBUF tiles ----
    xq = pers.tile([P, NT, P], BF16)          # [node_p, tile, 4b*32c]
    dxq = pers.tile([P, NT, P], BF16)
    xT2 = pers.tile([P, NB, NP], BF16)        # rows 0-31/64-95 x^T; 32-63/96-127 AG@x^T
    ETf = pers.tile([P, NP], F32)
    ETb = pers.tile([P, NP], BF16)
    TDT = pers.tile([P, NP], BF16)
    TWT = pers.tile([P, NP], BF16)
    AS = pers.tile([P, NT, NP], BF16)         # AS row-tiles
    ET64 = pers.tile([P, NT, NP], BF16)       # chunk c: [E(2c);Ed(2c);E(2c+1);Ed(2c+1)] x32
    E_cat = pers.tile([64, NP], BF16)         # rows 0-15 E^T, 32-47 (E*d)^T
    hoT = pers.tile([P, NP], BF16)
    h1 = pers.tile([P, NP], BF16)
    h2 = pers.tile([P, NP], BF16)
    Et = pers.tile([P, NP], BF16)
    EtDF = pers.tile([P, NP], BF16)
    DET = pers.tile([P, NP], BF16)
    dgT = pers.tile([32, NB, NP], BF16)       # x_dg2^T per batch at base 0
    agT = pers.tile([P, 2, NP], F32)          # final out^T per batch-pair
    WpT = pers.tile([P, NT, 64], BF16)
    fc1T = pers.tile([P, 32], BF16)
    fc2T = pers.tile([P, 16], BF16)
    fc3T = pers.tile([P, 16], BF16)
    b1p = pers.tile([P, 1], F32)
    b2p = pers.tile([P, 1], F32)
    b3p = pers.tile([P, 1], F32)
    abias = pers.tile([16, 64], F32)
    bWdg4 = pers.tile([P, 64], BF16)
    I128f = pers.tile([P, P], F32)
    I128b = pers.tile([P, P], BF16)
    ones_b = pers.tile([P, 16], BF16)         # ones, used as lhsT for rowsum/d16
    d_row = pers.tile([1, NP], F32)
    d_col = pers.tile([P, NT], F32)
    dmask = pers.tile([P, NT], F32)
    magic = pers.tile([P, NT], I32)

    # ---- input DMAs ----
    nc.sync.dma_start(out=xq, in_=t["xq"].ap())
    nc.sync.dma_start(out=xT2, in_=t["xT2"].ap())
    nc.sync.dma_start(out=ETf, in_=t["ETf"].ap())
    nc.sync.dma_start(out=ETb, in_=t["ETb"].ap())
    nc.sync.dma_start(out=TDT, in_=t["TDT"].ap())
    nc.sync.dma_start(out=TWT, in_=t["TWT"].ap())
    nc.sync.dma_start(out=WpT, in_=t["WpT"].ap())
    nc.sync.dma_start(out=fc1T, in_=t["fc1T"].ap())
    nc.sync.dma_start(out=fc2T, in_=t["fc2T"].ap())
    nc.sync.dma_start(out=fc3T, in_=t["fc3T"].ap())
    nc.sync.dma_start(out=b1p, in_=t["b1p"].ap())
    nc.sync.dma_start(out=b2p, in_=t["b2p"].ap())
    nc.sync.dma_start(out=b3p, in_=t["b3p"].ap())
    nc.sync.dma_start(out=abias, in_=t["abias"].ap())
    nc.sync.dma_start(out=bWdg4, in_=t["bWdg4"].ap())
    nc.sync.dma_start(out=dmask, in_=t["dmask"].ap())
    nc.sync.dma_start(out=E_cat[0:16, :], in_=t["ETb"].ap()[0:16, :])

    make_identity(nc, I128f)
    make_identity(nc, I128b)
    nc.vector.memset(ones_b, 1.0)
    nc.vector.memset(magic, RSQRT_MAGIC)

    out_d = t["out"]

    # ================= Phase 1: AS = relu(tanh(3 E E^T)) =================
    for g in range(2):
        for j in range(4):
            mt = 4 * g + j
            ps = ps_b.tile([P, NP], F32, tag="big")
            for r in range(2):
                nc.tensor.matmul(
                    ps[:, r * 512:(r + 1) * 512],
                    lhsT=ETf[32 * j:32 * j + 16, mt * P:(mt + 1) * P],
                    rhs=ETf[32 * j:32 * j + 16, r * 512:(r + 1) * 512],
                    start=True, stop=True,
                    tile_position=(32 * j, 0),
                )
            nc.scalar.activation(AS[:, mt, :], ps, AF.Tanh, scale=ALPHA)
            nc.vector.tensor_scalar_max(AS[:, mt, :], AS[:, mt, :], 0.0)

    # d = rowsum(AS)^-1/2 (row form via ones-lhsT matmuls, then transpose)
    dr_ps = ps_b.tile([1, NP], F32, tag="big")
    for kt in range(NT):
        for r in range(2):
            nc.tensor.matmul(
                dr_ps[0:1, r * 512:(r + 1) * 512],
                lhsT=ones_b[:, 0:1],
                rhs=AS[:, kt, r * 512:(r + 1) * 512],
                start=(kt == 0), stop=(kt == NT - 1),
            )
    nc.vector.tensor_copy(d_row, dr_ps)  # rowsums, f32 sbuf
    dc_ps = ps_b.tile([P, NT], F32, tag="big")
    for mt in range(NT):
        nc.tensor.transpose(
            dc_ps[:, mt:mt + 1], in_=d_row[0:1, mt * P:(mt + 1) * P],
            identity=I128f[0:1, 0:1],
        )
    # rsqrt via magic-number + 2 Newton iterations (avoids ACT table switch)
    s_sb = work.tile([P, NT], F32, tag="dtmp")
    nc.vector.tensor_scalar_max(s_sb, dc_ps, 1e-6)
    sh = work.tile([P, NT], I32, tag="dtmp_i")
    nc.vector.tensor_scalar(sh, s_sb.bitcast(I32), 1, 0, OP.logical_shift_right, OP.bypass)
    y0 = work.tile([P, NT], I32, tag="dtmp_y")
    nc.vector.tensor_tensor(y0, magic, sh, OP.subtract)
    yf = y0.bitcast(F32)
    cur = yf
    for it in range(2):
        t1 = work.tile([P, NT], F32, tag=f"nt1_{it}")
        nc.vector.tensor_tensor(t1, cur, cur, OP.mult)
        nc.vector.tensor_tensor(t1, t1, s_sb, OP.mult)
        nc.vector.tensor_scalar(t1, t1, -0.5, 1.5, OP.mult, OP.add)
        t2 = work.tile([P, NT], F32, tag=f"nt2_{it}")
        nc.vector.tensor_tensor(t2, cur, t1, OP.mult)
        cur = t2
    nc.vector.tensor_tensor(d_col, cur, dmask, OP.mult)  # mask kills padded nodes

    # dxq = d * x (token-major); diag_d -> d16 row-broadcast; E_cat[16:32] = E^T * d
    d16_ps = ps_b.tile([16, NP], F32, tag="big")
    for mt in range(NT):
        nc.vector.tensor_scalar_mul(dxq[:, mt, :], xq[:, mt, :], d_col[:, mt:mt + 1])
        dg_t = work.tile([P, P], BF16, tag="diag")
        nc.vector.tensor_scalar_mul(dg_t, I128b, d_col[:, mt:mt + 1])
        nc.tensor.matmul(
            d16_ps[:, mt * P:(mt + 1) * P], lhsT=ones_b, rhs=dg_t,
            start=True, stop=True,
        )
    nc.vector.tensor_tensor(E_cat[32:48, :], ETb[0:16, :], d16_ps, OP.mult)

    # ET64 chunks via broadcast DMAs (DRAM round-trip: SBUF APs need nonzero
    # partition step, DRAM APs don't)
    ecat_d = dram.tile([64, NP], BF16)
    nc.sync.dma_start(out=ecat_d, in_=E_cat)
    for c in range(NT):
        for seg, row in ((0, 2 * c), (32, 32 + 2 * c), (64, 2 * c + 1), (96, 33 + 2 * c)):
            src = ecat_d[row:row + 1, :]
            b_ap = bass.AP(tensor=src.tensor, offset=src.offset,
                           ap=[[0, 32]] + list(src.ap)[1:])
            nc.sync.dma_start(out=ET64[seg:seg + 32, c, :], in_=b_ap)

    if PHASES < 2:
        probe = work.tile([P, NT, 64], F32, tag="probe")
        for mt in range(NT):
            nc.vector.tensor_copy(probe[:, mt, :], AS[:, mt, 0:64])
        for nt_i in range(NT):
            nc.sync.dma_start(out=out_d.ap()[0, nt_i * 125:nt_i * 125 + 125, :],
                              in_=probe[0:125, nt_i, :])
        nc.sync.dma_start(out=out_d.ap()[1, 0:128, 0:8],
                          in_=d_col)
        p2 = work.tile([P, 64], F32, tag="probe2")
        nc.vector.tensor_copy(p2, ET64[:, 0, 0:64])
        nc.sync.dma_start(out=out_d.ap()[1, 128:256, 0:64], in_=p2)
        p3 = work.tile([P, 64], F32, tag="probe3")
        nc.vector.tensor_copy(p3, dxq[:, 0, 0:64])
        nc.sync.dma_start(out=out_d.ap()[1, 256:384, 0:64], in_=p3)
        return

    # ============ Phase 2: ho^T and raw AG-conv (AS @ (d*x))^T ============
    ho_ps = ps_a.tile([P, NP], F32, tag="acc1")
    xg_ps = ps_a.tile([P, NP], F32, tag="acc2")
    for vt in range(NT):
        for j in range(NB):
            for r in range(2):
                nc.tensor.matmul(
                    ho_ps[32 * j:32 * j + 32, r * 512:(r + 1) * 512],
                    lhsT=xq[:, vt, 32 * j:32 * j + 32],
                    rhs=AS[:, vt, r * 512:(r + 1) * 512],
                    start=(vt == 0), stop=(vt == NT - 1),
                    tile_position=(0, 32 * j),
                )
                nc.tensor.matmul(
                    xg_ps[32 * j:32 * j + 32, r * 512:(r + 1) * 512],
                    lhsT=dxq[:, vt, 32 * j:32 * j + 32],
                    rhs=AS[:, vt, r * 512:(r + 1) * 512],
                    start=(vt == 0), stop=(vt == NT - 1),
                    tile_position=(0, 32 * j),
                )
    nc.vector.tensor_copy(hoT, ho_ps)
    for j in range(NB):
        nc.vector.tensor_copy(xT2[32:64, j, :], xg_ps[32 * j:32 * j + 32, :])
        nc.sync.dma_start(out=xT2[96:128, j, :], in_=xT2[32:64, j, :])

    if PHASES < 3:
        p2 = work.tile([P, 64], F32, tag="probe2")
        nc.vector.tensor_copy(p2, hoT[:, 0:64])
        nc.sync.dma_start(out=out_d.ap()[0, 0:128, :], in_=p2)
        p3 = work.tile([P, 64], F32, tag="probe3")
        nc.vector.tensor_copy(p3, xT2[:, 0, 0:64])
        nc.sync.dma_start(out=out_d.ap()[1, 0:128, :], in_=p3)
        return

    # ===================== Phase 3: MLP + DE^T =====================
    m1_ps = ps_b.tile([P, NP], F32, tag="big")
    for j in range(NB):
        for r in range(2):
            nc.tensor.matmul(
                m1_ps[32 * j:32 * j + 32, r * 512:(r + 1) * 512],
                lhsT=fc1T[32 * j:32 * j + 32, :],
                rhs=hoT[32 * j:32 * j + 32, r * 512:(r + 1) * 512],
                start=True, stop=True, tile_position=(32 * j, 32 * j),
            )
    nc.scalar.activation(h1, m1_ps, AF.Sigmoid, bias=b1p[:, 0:1])
    m2_ps = ps_b.tile([P, NP], F32, tag="big")
    for j in range(NB):
        for r in range(2):
            nc.tensor.matmul(
                m2_ps[32 * j:32 * j + 16, r * 512:(r + 1) * 512],
                lhsT=fc2T[32 * j:32 * j + 32, :],
                rhs=h1[32 * j:32 * j + 32, r * 512:(r + 1) * 512],
                start=True, stop=True, tile_position=(32 * j, 32 * j),
            )
    nc.scalar.activation(h2, m2_ps, AF.Sigmoid, bias=b2p[:, 0:1])
    m3_ps = ps_b.tile([P, NP], F32, tag="big")
    for j in range(NB):
        for r in range(2):
            nc.tensor.matmul(
                m3_ps[32 * j:32 * j + 16, r * 512:(r + 1) * 512],
                lhsT=fc3T[32 * j:32 * j + 16, :],
                rhs=h2[32 * j:32 * j + 16, r * 512:(r + 1) * 512],
                start=True, stop=True, tile_position=(32 * j, 32 * j),
            )
    nc.vector.tensor_tensor(Et, TDT, TWT, OP.mult)
    nc.vector.tensor_tensor(Et, Et, ETb, OP.mult)
    # EtDF = (DF + b3) * Et ; DE^T = tanh(3 EtDF)
    nc.vector.scalar_tensor_tensor(EtDF, m3_ps, b3p[:, 0:1], Et, OP.add, OP.mult)
    nc.scalar.activation(DET, EtDF, AF.Tanh, scale=ALPHA)

    if PHASES < 4:
        p2 = work.tile([P, 64], F32, tag="probe2")
        nc.vector.tensor_copy(p2, DET[:, 0:64])
        nc.sync.dma_start(out=out_d.ap()[0, 0:128, :], in_=p2)
        return

    # ========== Phase 4: S/DA (flash-style) + x_dg2 accumulation ==========
    dg_ps = ps_a.tile([P, NP], F32, tag="acc1")
    for mt in range(NT):
        for pair in range(2):
            for bb in range(2):
                j = 2 * pair + bb
                s_ps = ps_b.tile([P, NP], F32, tag="big")
                for r in range(2):
                    nc.tensor.matmul(
                        s_ps[:, r * 512:(r + 1) * 512],
                        lhsT=DET[32 * j:32 * j + 16, mt * P:(mt + 1) * P],
                        rhs=DET[32 * j:32 * j + 16, r * 512:(r + 1) * 512],
                        start=True, stop=True, tile_position=(32 * j, 0),
                    )
                da_t = da_p.tile([P, NP], BF16, tag="da")
                nc.scalar.activation(da_t, s_ps, AF.Tanh, scale=ALPHA)
                nc.vector.tensor_scalar_max(da_t, da_t, 0.0)
                for r in range(2):
                    nc.tensor.matmul(
                        dg_ps[32 * j:32 * j + 32, r * 512:(r + 1) * 512],
                        lhsT=xq[:, mt, 32 * j:32 * j + 32],
                        rhs=da_t[:, r * 512:(r + 1) * 512],
                        start=(mt == 0), stop=(mt == NT - 1),
                        tile_position=(0, 32 * j),
                    )
    for j in range(NB):
        nc.vector.tensor_copy(dgT[:, j, :], dg_ps[32 * j:32 * j + 32, :])

    if PHASES < 5:
        p2 = work.tile([P, 64], F32, tag="probe2")
        nc.vector.tensor_copy(p2, dgT[:, 0:64])
        nc.sync.dma_start(out=out_d.ap()[0, 0:128, :], in_=p2)
        return

    # ====== Phase 5: agconv y^T tt + big contraction + bias + dg fold ======
    SUB = int(os.environ.get("KERNEL_SUB", "3"))
    for pair in range(2):
        ag_ps = ps_a.tile([P, 2, 512], F32, tag="acc2")
        for c in range(NT):
            for bb in range(2):
                j = 2 * pair + bb
                yt = yt_p.tile([P, NP], BF16, tag="yt")
                nc.vector.tensor_tensor(yt, xT2[:, j, :], ET64[:, c, :], OP.mult)
                if SUB < 1:
                    continue
                for tch in range(2):
                    nc.tensor.matmul(
                        ag_ps[64 * bb:64 * bb + 64, tch, :],
                        lhsT=WpT[:, c, :],
                        rhs=yt[:, tch * 512:(tch + 1) * 512],
                        start=(c == 0), stop=(SUB == 1 and c == NT - 1),
                        tile_position=(0, 64 * bb),
                        skip_group_check=True,
                    )
        if SUB < 1:
            continue
        if SUB >= 2:
            for bb in range(2):
                j = 2 * pair + bb
                for tch in range(2):
                    nc.tensor.matmul(
                        ag_ps[64 * bb:64 * bb + 64, tch, :],
                        lhsT=abias, rhs=ETf[0:16, tch * 512:(tch + 1) * 512],
                        start=False, stop=(SUB == 2),
                        tile_position=(0, 64 * bb),
                        skip_group_check=True,
                    )
                    if SUB >= 3:
                        nc.tensor.matmul(
                            ag_ps[64 * bb:64 * bb + 64, tch, :],
                            lhsT=bWdg4[0:32, :],
                            rhs=dgT[:, j, tch * 512:(tch + 1) * 512],
                            start=False, stop=True,
                            tile_position=(0, 64 * bb),
                            skip_group_check=True,
                        )
        nc.vector.tensor_copy(agT[:, pair, :].rearrange("p (a b) -> p a b", a=2),
                              ag_ps)
    if SUB < 1:
        return

    if PHASES < 6:
        for pair in range(2):
            nc.sync.dma_start(out=out_d.ap()[pair, 0:128, :], in_=agT[:, pair, 0:64])
        return

    # ============== Phase 6: transpose to token-major + DMA out ==============
    for b in range(NB):
        pair, bb = b // 2, b % 2
        tr = ps_a.tile([P, NT, 64], F32, tag="acc1")
        for nt_i in range(NT):
            nc.tensor.transpose(
                tr[:, nt_i, :],
                in_=agT[64 * bb:64 * bb + 64, pair, nt_i * P:(nt_i + 1) * P],
                identity=I128f[64 * bb:64 * bb + 64, 64 * bb:64 * bb + 64],
            )
        tr_sb = work.tile([P, NT, 64], F32, tag="tr_sb")
        nc.vector.tensor_copy(tr_sb, tr)
        for nt_i in range(NT):
            rows = min(P, N - nt_i * P)
            nc.sync.dma_start(
                out=out_d.ap()[b, nt_i * P:nt_i * P + rows, :],
                in_=tr_sb[0:rows, nt_i, :],
            )


def _build_nc():
    nc = bacc.Bacc("TRN2", target_bir_lowering=False, debug=False,
                   num_devices=NCORES)
    P = 128
    t = {}
    t["xq"] = nc.dram_tensor("xq", [P, NT, P], BF16, kind="ExternalInput")
    t["xT2"] = nc.dram_tensor("xT2", [P, NB, NP], BF16, kind="ExternalInput")
    t["ETf"] = nc.dram_tensor("ETf", [P, NP], F32, kind="ExternalInput")
    t["ETb"] = nc.dram_tensor("ETb", [P, NP], BF16, kind="ExternalInput")
    t["TDT"] = nc.dram_tensor("TDT", [P, NP], BF16, kind="ExternalInput")
    t["TWT"] = nc.dram_tensor("TWT", [P, NP], BF16, kind="ExternalInput")
    t["WpT"] = nc.dram_tensor("WpT", [P, NT, 64], BF16, kind="ExternalInput")
    t["fc1T"] = nc.dram_tensor("fc1T", [P, 32], BF16, kind="ExternalInput")
    t["fc2T"] = nc.dram_tensor("fc2T", [P, 16], BF16, kind="ExternalInput")
    t["fc3T"] = nc.dram_tensor("fc3T", [P, 16], BF16, kind="ExternalInput")
    t["b1p"] = nc.dram_tensor("b1p", [P, 1], F32, kind="ExternalInput")
    t["b2p"] = nc.dram_tensor("b2p", [P, 1], F32, kind="ExternalInput")
    t["b3p"] = nc.dram_tensor("b3p", [P, 1], F32, kind="ExternalInput")
    t["abias"] = nc.dram_tensor("abias", [16, 64], F32, kind="ExternalInput")
    t["bWdg4"] = nc.dram_tensor("bWdg4", [P, 64], BF16, kind="ExternalInput")
    t["dmask"] = nc.dram_tensor("dmask", [P, NT], F32, kind="ExternalInput")
    t["out"] = nc.dram_tensor("out", [NB, N, 64], F32, kind="ExternalOutput")

    with tile.TileContext(nc) as tc:
        with ExitStack() as ctx:
            _build_body(nc, tc, ctx, t)
    nc.finalize()
    return nc


def _prep_core_inputs(core, x, E, TD, TW, Wp, bp, Wdg, a, b,
                      fc1_w, fc1_b, fc2_w, fc2_b, fc3_w, fc3_b):
    P = 128
    bs = slice(NB * core, NB * (core + 1))
    xp = np.zeros((NB, NP, 32), np.float32)
    xp[:, :N] = x[bs]
    Ep = np.zeros((NP, 16), np.float32)
    Ep[:N] = E

    xq = np.zeros((P, NT, P), np.float32)
    for ti in range(NT):
        blk = xp[:, ti * P:(ti + 1) * P, :]          # [4,128,32]
        xq[:, ti, :] = blk.transpose(1, 0, 2).reshape(P, P)
    xT2 = np.zeros((P, NB, NP), np.float32)
    xT = xp.transpose(2, 0, 1)                        # [32, 4, 1024]
    xT2[0:32] = xT
    xT2[64:96] = xT

    ETf = np.zeros((P, NP), np.float32)
    TDT = np.zeros((P, NP), np.float32)
    TWT = np.zeros((P, NP), np.float32)
    fc1T = np.zeros((P, 32), np.float32)
    fc2T = np.zeros((P, 16), np.float32)
    fc3T = np.zeros((P, 16), np.float32)
    b1p = np.zeros((P, 1), np.float32)
    b2p = np.zeros((P, 1), np.float32)
    b3p = np.zeros((P, 1), np.float32)
    bWdg4 = np.zeros((P, 64), np.float32)
    for j in range(NB):
        r0 = 32 * j
        ETf[r0:r0 + 16] = Ep.T
        TDT[r0:r0 + 16, :N] = TD[NB * core + j].T
        TWT[r0:r0 + 16, :N] = TW[NB * core + j].T
        fc1T[r0:r0 + 32] = fc1_w.T
        fc2T[r0:r0 + 32] = fc2_w.T
        fc3T[r0:r0 + 16] = fc3_w.T
        b1p[r0:r0 + 32, 0] = fc1_b
        b2p[r0:r0 + 16, 0] = fc2_b
        b3p[r0:r0 + 16, 0] = fc3_b
        bWdg4[r0:r0 + 32] = b * Wdg

    dmask_h = np.zeros((P, NT), np.float32)
    for mt in range(NT):
        for p in range(P):
            dmask_h[p, mt] = 1.0 if mt * P + p < N else 0.0

    WpT = np.zeros((P, NT, 64), np.float32)
    for c in range(NT):
        for h in range(2):
            d = 2 * c + h
            WpT[64 * h:64 * h + 32, c, :] = a * Wp[d, 0]
            WpT[64 * h + 32:64 * h + 64, c, :] = a * Wp[d, 1]

    return {
        "xq": xq.astype(bfloat16),
        "xT2": xT2.astype(bfloat16),
        "ETf": ETf,
        "ETb": ETf.astype(bfloat16),
        "TDT": TDT.astype(bfloat16),
        "TWT": TWT.astype(bfloat16),
        "WpT": WpT.astype(bfloat16),
        "fc1T": fc1T.astype(bfloat16),
        "fc2T": fc2T.astype(bfloat16),
        "fc3T": fc3T.astype(bfloat16),
        "b1p": b1p, "b2p": b2p, "b3p": b3p,
        "abias": (a * bp).astype(np.float32),
        "bWdg4": bWdg4.astype(bfloat16),
        "dmask": dmask_h,
    }


def kernel(x, E_id_emb, T_D_emb, T_W_emb, weights_pool, bias_pool, Wdg, a, b,
           fc1_w, fc1_b, fc2_w, fc2_b, fc3_w, fc3_b):
    global LAST_EXEC_NS
    from concourse.bass_utils import run_bass_kernel_spmd

    x = np.asarray(x, np.float32)
    E = np.asarray(E_id_emb, np.float32)
    TD = np.asarray(T_D_emb, np.float32)
    TW = np.asarray(T_W_emb, np.float32)
    Wp = np.asarray(weights_pool, np.float32)
    bp = np.asarray(bias_pool, np.float32)
    Wdg_ = np.asarray(Wdg, np.float32)
    a_ = float(np.asarray(a).reshape(-1)[0])
    b_ = float(np.asarray(b).reshape(-1)[0])
    args = (x, E, TD, TW, Wp, bp, Wdg_, a_, b_,
            np.asarray(fc1_w, np.float32), np.asarray(fc1_b, np.float32),
            np.asarray(fc2_w, np.float32), np.asarray(fc2_b, np.float32),
            np.asarray(fc3_w, np.float32), np.asarray(fc3_b, np.float32))

    key = os.environ.get("KERNEL_PHASES", "6") + "." + os.environ.get("KERNEL_SUB", "3")
    if key not in _NC_CACHE:
        _NC_CACHE[key] = _build_nc()
    nc = _NC_CACHE[key]

    in_maps = [_prep_core_inputs(c, *args) for c in range(NCORES)]
    trace = bool(int(os.environ.get("BASS_KERNEL_TRACE", "0")))
    if trace:
        try:
            import profile_hook  # noqa: F401
        except ImportError:
            pass
    res = run_bass_kernel_spmd(nc, in_maps, core_ids=list(range(NCORES)),
                               trace=trace)
    LAST_EXEC_NS = res.exec_time_ns
    out = np.concatenate([res.results[c]["out"] for c in range(NCORES)], axis=0)
    return np.ascontiguousarray(out.astype(np.float32))
